# revision 53
# baseline (speedup 1.0000x reference)
"""Trainium2 Bass kernel for the 13-branch scattering-GAT network.

Strategy (8 NeuronCores, row-parallel, v3):
  - Nodes sharded 512/core. All constant inputs host-prewrapped to
    partition-major [128, X] contiguous layouts so every load is a fat DMA.
  - psi streamed per 2-ktile chunk on alternating sync/scalar DMA queues
    (read twice: level-1 and level-2 wavelets), freeing SBUF for gather
    buffers. A tiny warmup AllGather absorbs the CC first-trigger latency.
  - Three AllGathers (|y1|, |y2|, h||es) write Shared-scratchpad DRAM.
  - Edge softmax-aggregation: per-edge rows of the AllGathered h table are
    fetched with prepare_only dma_gather. Descriptors for 8 frames (SWDGE
    ring depth: 1 per lane x 8 lanes) are generated on GpSimd during the
    wavelet phases and fired by one trigger_dma right after the h-AG
    (explicit sync dep; the gathers read an address alias so Tile cannot
    invert the dependency). Remaining frames prep behind trigger1/w0
    readers. Per-lane hg_dma sems replace Tile's un-incremented DMASW
    waits via _patch_prep_waits.
  - ed[dst] lookup per edge is a small PE matmul against a host-shipped
    transposed 0/1 indicator (runs inside the h-AG window). Aggregation is
    dense 128-edge-tile matmuls into PSUM; the exp(leaky) edge-weight
    expand-multiply is split DVE-1x / Scalar-expand + DVE-2x (bf16 packed).
  - Self-loops folded in locally; per-window MLP + head run inside the
    edge loop (fills gather-paced slack); batched log_softmax at the end.
"""

import sys

sys.path.insert(0, "/opt/trn_rl_repo")

import numpy as np
import ml_dtypes

import concourse.bass as bass
import concourse.mybir as mybir
import concourse.tile as tile
from concourse import bacc
from concourse.bass import _add_dep_helper
from concourse.bass_utils import run_bass_kernel_spmd


def _add_dep(dependent, dependency, sync, reason):
    _add_dep_helper(dependent, dependency, sync=sync, reason=reason)

R = 8          # cores
N = 4096       # nodes
S = N // R     # nodes per core (512)
F = 32         # features
H = 2          # heads
G = 13         # branches
GH = G * H     # 26
NHID = 64
C = 10
J = 3
KT = N // 128  # 32 contraction tiles
NW = S // 128  # 4 dst windows per core
HROW = 896     # padded AG row width (1792B, 256B-aligned)
EW = GH * 33   # 858 edge-matmul output width per dst window
NEG = 0.2
CHK = 6        # ktiles per rhs-build chunk

BF = mybir.dt.bfloat16
F32 = mybir.dt.float32
I16 = mybir.dt.int16

_bf = lambda a: np.ascontiguousarray(a.astype(ml_dtypes.bfloat16))
_f32 = lambda a: np.ascontiguousarray(a.astype(np.float32))

_PROGRAM_CACHE = {}


def _wrap128(a):
    """[KT*128, X] -> [128, KT*X] partition-major."""
    n, x = a.shape
    k = n // 128
    return np.ascontiguousarray(a.reshape(k, 128, x).transpose(1, 0, 2).reshape(128, k * x))


def build_program(KMT):
    TE = NW * KMT            # total edge k-tiles
    KE = TE * 128            # padded edge count
    nc = bacc.Bacc("TRN2", target_bir_lowering=False, debug=False, num_devices=R)

    # ---------------- I/O (all host-prewrapped partition-major) -------------
    d_af = nc.dram_tensor("af", [128, KT * F], BF, kind="ExternalInput")
    d_psi = nc.dram_tensor("psiW", [128, KT * J * S], BF, kind="ExternalInput")
    d_u = nc.dram_tensor("uW", [128, KT * S], BF, kind="ExternalInput")
    d_wcat = nc.dram_tensor("wcat", [F, G * 68], BF, kind="ExternalInput")
    d_bias = nc.dram_tensor("bias", [128, G * H * F], F32, kind="ExternalInput")
    d_mw = nc.dram_tensor("mw", [128, 7 * 128], BF, kind="ExternalInput")
    d_mbp = nc.dram_tensor("mbp2", [128, 7], F32, kind="ExternalInput")
    d_outw = nc.dram_tensor("outwW", [128, 7 * C], BF, kind="ExternalInput")
    d_gidx = nc.dram_tensor("gidx", [128, KE // 16], I16, kind="ExternalInput")
    d_indF = nc.dram_tensor("indF", [128, TE * 128], BF, kind="ExternalInput")
    d_indT = nc.dram_tensor("indT", [128, TE * 128], BF, kind="ExternalInput")
    d_out = nc.dram_tensor("out", [S, C], F32, kind="ExternalOutput")

    from concourse.masks import make_identity

    with tile.TileContext(nc) as tc:
        with (
            tc.tile_pool(name="const", bufs=1) as kc,
            tc.tile_pool(name="work", bufs=1) as wk,
        ):
            # ---------------- constant loads ----------------
            af_sb = kc.tile([128, KT * F], BF)
            nc.sync.dma_start(af_sb[:], d_af[:])
            gidx_sb = kc.tile([128, KE // 16], I16)
            nc.scalar.dma_start(gidx_sb[:], d_gidx[:])
            wcat_sb = kc.tile([F, G * 68], BF)
            nc.scalar.dma_start(wcat_sb[:], d_wcat[:])
            bias_sb = kc.tile([128, G * H * F], F32)
            nc.scalar.dma_start(bias_sb[:], d_bias[:])
            mw_sb = kc.tile([128, 7 * 128], BF)
            nc.scalar.dma_start(mw_sb[:], d_mw[:])
            mbp2_sb = kc.tile([128, 7], F32)
            nc.scalar.dma_start(mbp2_sb[:], d_mbp[:])
            outw_sb = kc.tile([128, 7 * C], BF)
            nc.scalar.dma_start(outw_sb[:], d_outw[:])

            ident = kc.tile([128, 128], BF)
            make_identity(nc, ident[:])
            identf = kc.tile([128, 128], F32)
            make_identity(nc, identf[:])

            # tiny warmup AllGather issued at t~0: absorbs the CC-stack
            # first-trigger latency (~11us) so AG-a1 triggers fast
            warm_in = nc.dram_tensor("warm_in", [1, 64], BF, kind="Internal")
            warm_out = nc.dram_tensor("warm_out", [R, 64], BF, kind="Internal",
                                      addr_space="Shared")
            nc.gpsimd.collective_compute(
                "AllGather", mybir.AluOpType.bypass,
                replica_groups=[list(range(R))],
                ins=[warm_in[:].opt()], outs=[warm_out[:].opt()],
            )

            # ---------------- persistent work tiles ----------------
            indT_sb = wk.tile([128, TE * 128], BF)
            indF_sb = wk.tile([128, TE * 128], BF)
            hes_sb = wk.tile([128, NW * HROW], BF)
            nc.vector.memset(
                hes_sb[:].rearrange("p (m c) -> p m c", c=HROW)[:, :, EW:HROW], 0.0)
            esloc = wk.tile([128, NW * GH], F32)
            edloc = wk.tile([128, NW * GH], F32)
            edlocB = wk.tile([128, NW * GH], BF)
            # 3 rotating whole-window gather buffers
            hg_bufs = [wk.tile([128, KMT * HROW], BF, tag="hg", bufs=3,
                               name=f"hg{i}") for i in range(3)]

            # DRAM: AG staging (Local in, Shared out)
            aga1_in = nc.dram_tensor("aga1_in", [S, J * F], BF, kind="Internal")
            aga1_out = nc.dram_tensor("aga1_out", [N, J * F], BF, kind="Internal")
            aga2_in = nc.dram_tensor("aga2_in", [S, J * J * F], BF, kind="Internal")
            aga2_out = nc.dram_tensor("aga2_out", [N, J * J * F], BF, kind="Internal", addr_space="Shared")
            agh_in = nc.dram_tensor("agh_in", [S, HROW], BF, kind="Internal")
            agh_out = nc.dram_tensor("agh_out", [N, HROW], BF, kind="Internal", addr_space="Shared")
            # alias of agh_out for the prepared gathers: hides the read from
            # Tile's dep tracker (else the AG inherits a WAR wait on gather
            # DMAs that only fire post-AG -> deadlock). Ordering is restored
            # by an explicit sync dep from trigger_dma onto the h-AG
            # instruction (Tile emits the Collectives-tick wait from it).
            agh_rd = nc.dram_tensor("agh_rd", [N, HROW], BF, kind="Internal", addr_space="Shared")
            nc.lookup_mloc(agh_rd).addr = nc.lookup_mloc(agh_out).addr
            rg = [list(range(R))]
            # one DMA-completion sem per SWDGE lane (8 lanes, round-robin in
            # scheduled order = emission order); exact lane-FIFO semantics
            hg_sems = [nc.alloc_semaphore(f"hg_dma{i}") for i in range(8)]
            _prep_ctr = [0]

            def gather_window(w, chunks=None):
                # prepare_only: descriptors generated on GpSimd NOW (off the
                # post-AG critical path); the DMA fires at the next
                # trigger_dma. chunked <=768 idxs per call (ISA limit).
                # SWDGE ring holds 1 outstanding frame per lane (8 total):
                # at most 8 preps may be pending before the first trigger.
                hg = hg_bufs[w % 3]
                ncks = (KMT + CHK - 1) // CHK
                out = []
                for c in range(ncks) if chunks is None else chunks:
                    t0 = c * CHK
                    nk = min(CHK, KMT - t0)
                    j = _prep_ctr[0]
                    _prep_ctr[0] += 1
                    out.append(nc.gpsimd.dma_gather(
                        out_ap=hg[:, t0 * HROW:(t0 + nk) * HROW]
                        .rearrange("p (c x) -> p c x", x=HROW),
                        in_ap=agh_rd[:],
                        idxs_ap=gidx_sb[:, (w * KMT + t0) * 8:
                                        (w * KMT + t0 + nk) * 8],
                        num_idxs=nk * 128,
                        num_idxs_reg=nk * 128,
                        elem_size=HROW,
                        prepare_only=True,
                        sem=hg_sems[j % 8],
                    ))
                return out

            # window-0 desc-gen early (GpSimd idle during phase 2)
            gather_window(0)


            # ============ phases 2-5: wavelet tree + coefs + GAT linear ======
            NPC = 2  # ktiles per psi chunk
            with tc.tile_pool(name="st", bufs=1) as st:
                a1_sb = st.tile([128, KT * J * F], BF)
                coefsT2 = st.tile([F, G * S], BF)
                a1T = st.tile([F, J * S], BF)
                a1loc = st.tile([128, NW * J * F], BF)
                a2T = st.tile([J * F, J * S], BF)
                a2loc = st.tile([128, NW * J * J * F], BF)

                # ---- phase 2: y1_j = psi_j @ af (psi streamed)
                with tc.tile_pool(name="psA2", bufs=1, space="PSUM") as psA2:
                    p_y1 = [psA2.tile([F, S], F32, tag=f"y1{j}", bufs=1,
                                      name=f"y1{j}") for j in range(J)]
                    for kc_ in range(KT // NPC):
                        psi_c = st.tile([128, NPC * J * S], BF, tag="psi", bufs=4)
                        (nc.sync if kc_ % 2 == 0 else nc.scalar).dma_start(
                            psi_c[:],
                            d_psi[:, kc_ * NPC * J * S:(kc_ + 1) * NPC * J * S])
                        for dk in range(NPC):
                            kt = kc_ * NPC + dk
                            for j in range(J):
                                nc.tensor.matmul(
                                    p_y1[j][:],
                                    lhsT=af_sb[:, kt * F:(kt + 1) * F],
                                    rhs=psi_c[:, dk * J * S + j * S:
                                              dk * J * S + (j + 1) * S],
                                    start=(kt == 0), stop=(kt == KT - 1),
                                )
                    for j in range(J):
                        nc.scalar.activation(
                            a1T[:, j * S:(j + 1) * S], p_y1[j][:],
                            mybir.ActivationFunctionType.Abs)
                    for j in range(J):
                        for mt in range(NW):
                            p_tp = psA2.tile([128, F], BF, tag="tp", bufs=2)
                            nc.tensor.transpose(
                                p_tp[:],
                                a1T[:, j * S + mt * 128: j * S + (mt + 1) * 128],
                                ident[:F, :F],
                            )
                            nc.vector.tensor_copy(
                                a1loc[:, mt * J * F + j * F: mt * J * F + (j + 1) * F],
                                p_tp[:],
                            )
                for mt in range(NW):
                    nc.sync.dma_start(
                        aga1_in[mt * 128:(mt + 1) * 128, :],
                        a1loc[:, mt * J * F:(mt + 1) * J * F],
                    )
                nc.gpsimd.collective_compute(
                    "AllGather", mybir.AluOpType.bypass, replica_groups=rg,
                    ins=[aga1_in[:].opt()], outs=[aga1_out[:].opt()],
                )
                # windows 1-2 desc-gen during the a1 AllGather / phase 3
                # (ring cap: at most 8 preps pending before trigger1)
                NCKS = (KMT + CHK - 1) // CHK
                W2E = max(0, min(NCKS, 8 - 2 * NCKS))
                gather_window(1)
                gather_window(2, chunks=tuple(range(W2E)))
                nc.scalar.dma_start(indT_sb[:], d_indT[:])
                nc.scalar.dma_start(
                    a1_sb[:].rearrange("p (k c) -> p k c", c=J * F),
                    aga1_out[:].rearrange("(k p) c -> p k c", p=128),
                )

                # ---- phase 4a: coef g=0 (af group) — runs during AG-a1
                with tc.tile_pool(name="psB", bufs=1, space="PSUM") as psB:
                    pA = psB.tile([F, S], F32, tag="cA", bufs=1, name="cA")
                    for kt4 in range(KT // 4):
                        u_c = st.tile([128, 4 * S], BF, tag="u", bufs=2)
                        nc.scalar.dma_start(
                            u_c[:], d_u[:, kt4 * 4 * S:(kt4 + 1) * 4 * S])
                        for dk in range(4):
                            kt = kt4 * 4 + dk
                            nc.tensor.matmul(
                                pA[:], lhsT=af_sb[:, kt * F:(kt + 1) * F],
                                rhs=u_c[:, dk * S:(dk + 1) * S],
                                start=(kt == 0), stop=(kt == KT - 1))
                    nc.vector.tensor_copy(coefsT2[:, 0:S], pA[:])

                # ---- phase 3: y2_{j,k} = psi_k @ a1_j (psi streamed again)
                with tc.tile_pool(name="psA3", bufs=1, space="PSUM") as psA3:
                    p_y2 = [psA3.tile([J * F, S], F32, tag=f"y2{k}", bufs=1,
                                      name=f"y2{k}") for k in range(J)]
                    for kc_ in range(KT // NPC):
                        psi_c = st.tile([128, NPC * J * S], BF, tag="psi", bufs=4)
                        (nc.sync if kc_ % 2 == 0 else nc.scalar).dma_start(
                            psi_c[:],
                            d_psi[:, kc_ * NPC * J * S:(kc_ + 1) * NPC * J * S])
                        for dk in range(NPC):
                            kt = kc_ * NPC + dk
                            for k in range(J):
                                nc.tensor.matmul(
                                    p_y2[k][:],
                                    lhsT=a1_sb[:, kt * J * F:(kt + 1) * J * F],
                                    rhs=psi_c[:, dk * J * S + k * S:
                                              dk * J * S + (k + 1) * S],
                                    start=(kt == 0), stop=(kt == KT - 1),
                                )
                    for k in range(J):
                        nc.scalar.activation(
                            a2T[:, k * S:(k + 1) * S], p_y2[k][:],
                            mybir.ActivationFunctionType.Abs)
                    for k in range(J):
                        for mt in range(NW):
                            p_tp2 = psA3.tile([128, J * F], BF, tag="tp2", bufs=2)
                            nc.tensor.transpose(
                                p_tp2[:],
                                a2T[:, k * S + mt * 128: k * S + (mt + 1) * 128],
                                ident[:J * F, :J * F],
                            )
                            nc.vector.tensor_copy(
                                a2loc[:, mt * J * J * F + k * J * F:
                                      mt * J * J * F + (k + 1) * J * F],
                                p_tp2[:],
                            )
                for mt in range(NW):
                    nc.sync.dma_start(
                        aga2_in[mt * 128:(mt + 1) * 128, :],
                        a2loc[:, mt * J * J * F:(mt + 1) * J * J * F],
                    )
                nc.gpsimd.collective_compute(
                    "AllGather", mybir.AluOpType.bypass, replica_groups=rg,
                    ins=[aga2_in[:].opt()], outs=[aga2_out[:].opt()],
                )

                # ---- phase 4a2: coef g=1..3 (a1 group) — runs during AG-a2
                with tc.tile_pool(name="psB1b", bufs=1, space="PSUM") as psB1b:
                    pB = psB1b.tile([J * F, S], F32, tag="cB", bufs=1, name="cB")
                    for kt4 in range(KT // 4):
                        u_c = st.tile([128, 4 * S], BF, tag="u", bufs=2)
                        nc.scalar.dma_start(
                            u_c[:], d_u[:, kt4 * 4 * S:(kt4 + 1) * 4 * S])
                        for dk in range(4):
                            kt = kt4 * 4 + dk
                            nc.tensor.matmul(
                                pB[:], lhsT=a1_sb[:, kt * J * F:(kt + 1) * J * F],
                                rhs=u_c[:, dk * S:(dk + 1) * S],
                                start=(kt == 0), stop=(kt == KT - 1))
                    for j in range(J):
                        nc.vector.tensor_copy(
                            coefsT2[:, (1 + j) * S:(2 + j) * S],
                            pB[j * F:(j + 1) * F, :])

                # ---- phase 4b: coef g=4..12 (a2 groups), u + a2 streamed
                with tc.tile_pool(name="psB2", bufs=1, space="PSUM") as psB2:
                    pC = psB2.tile([128, S], F32, tag="cC", bufs=1, name="cC")
                    pD = psB2.tile([128, S], F32, tag="cD", bufs=1, name="cD")
                    pE = psB2.tile([F, S], F32, tag="cE", bufs=1, name="cE")
                    for kt4 in range(KT // 4):
                        u_c = st.tile([128, 4 * S], BF, tag="u", bufs=2)
                        nc.scalar.dma_start(
                            u_c[:], d_u[:, kt4 * 4 * S:(kt4 + 1) * 4 * S])
                        a2_c = st.tile([128, 4 * J * J * F], BF, tag="a2c", bufs=2)
                        nc.sync.dma_start(
                            a2_c[:].rearrange("p (k c) -> p k c", c=J * J * F),
                            aga2_out[kt4 * 512:(kt4 + 1) * 512, :]
                            .rearrange("(k p) c -> p k c", p=128),
                        )
                        for dk in range(4):
                            kt = kt4 * 4 + dk
                            base = dk * J * J * F
                            nc.tensor.matmul(
                                pC[:], lhsT=a2_c[:, base:base + 4 * F],
                                rhs=u_c[:, dk * S:(dk + 1) * S],
                                start=(kt == 0), stop=(kt == KT - 1))
                            nc.tensor.matmul(
                                pD[:], lhsT=a2_c[:, base + 4 * F:base + 8 * F],
                                rhs=u_c[:, dk * S:(dk + 1) * S],
                                start=(kt == 0), stop=(kt == KT - 1))
                            nc.tensor.matmul(
                                pE[:], lhsT=a2_c[:, base + 8 * F:base + 9 * F],
                                rhs=u_c[:, dk * S:(dk + 1) * S],
                                start=(kt == 0), stop=(kt == KT - 1))
                    for ci in range(9):
                        j, k = ci % 3, ci // 3
                        g = 4 + 3 * j + k
                        srcp = (pC, pD, pE)[ci // 4]
                        off = (ci % 4) * F if ci < 8 else 0
                        nc.vector.tensor_copy(
                            coefsT2[:, g * S:(g + 1) * S], srcp[off:off + F, :])

                # ============ phase 5: h||es||ed = coefs @ wcat =============
                with tc.tile_pool(name="psB5", bufs=1, space="PSUM") as psB5:
                    for mt in range(NW):
                        p_h = psB5.tile([128, 1024], F32, tag="h5", bufs=2)
                        for g in range(G):
                            pc = g * 68 if g < 7 else 512 + (g - 7) * 68
                            nc.tensor.matmul(
                                p_h[:, pc:pc + 68],
                                lhsT=coefsT2[:, g * S + mt * 128: g * S + (mt + 1) * 128],
                                rhs=wcat_sb[:, g * 68:(g + 1) * 68],
                                start=True, stop=True,
                            )
                        for g0, ng in ((0, 7), (7, 6)):
                            base = 0 if g0 == 0 else 512
                            ph = p_h[:, base: base + ng * 68]\
                                .rearrange("p (g c) -> p g c", c=68)
                            nc.vector.tensor_copy(
                                hes_sb[:, mt * HROW + g0 * H * F:
                                       mt * HROW + (g0 + ng) * H * F]
                                .rearrange("p (g c) -> p g c", c=H * F),
                                ph[:, :, 0:H * F],
                            )
                            nc.vector.tensor_copy(
                                hes_sb[:, mt * HROW + G * H * F + g0 * H:
                                       mt * HROW + G * H * F + (g0 + ng) * H]
                                .rearrange("p (g c) -> p g c", c=H),
                                ph[:, :, H * F:H * F + H],
                            )
                            nc.vector.tensor_copy(
                                esloc[:, mt * GH + g0 * H: mt * GH + (g0 + ng) * H]
                                .rearrange("p (g c) -> p g c", c=H),
                                ph[:, :, H * F:H * F + H],
                            )
                            nc.vector.tensor_copy(
                                edloc[:, mt * GH + g0 * H: mt * GH + (g0 + ng) * H]
                                .rearrange("p (g c) -> p g c", c=H),
                                ph[:, :, H * F + H:H * F + 2 * H],
                            )
                            nc.vector.tensor_copy(
                                edlocB[:, mt * GH + g0 * H: mt * GH + (g0 + ng) * H]
                                .rearrange("p (g c) -> p g c", c=H),
                                ph[:, :, H * F + H:H * F + 2 * H],
                            )
                        nc.sync.dma_start(
                            agh_in[mt * 128:(mt + 1) * 128, :],
                            hes_sb[:, mt * HROW:(mt + 1) * HROW],
                        )
            # s1T outlives the edge phase (built per-window inside it)
            with tc.tile_pool(name="wk2", bufs=1) as wk2:
                s1T = wk2.tile([128, 7 * S], BF)
                nc.vector.memset(s1T[64:128, 6 * S:7 * S], 0.0)
                lg = wk2.tile([C, S], F32)
                # indF load lands in the h-AG window (sync queue idle);
                # indT loads earlier (after AG-a1) for the edp matmuls
                nc.sync.dma_start(indF_sb[:], d_indF[:])
                cc_h = nc.gpsimd.collective_compute(
                    "AllGather", mybir.AluOpType.bypass, replica_groups=rg,
                    ins=[agh_in[:].opt()], outs=[agh_out[:].opt()],
                )
                # fire the 8 prepared gathers (w0, w1, w2c0-1). The preps
                # read agh_rd (untracked alias), so the h-AG ordering is
                # restored with an explicit sync dep: Tile schedules the
                # trigger after the AG and waits its Collectives tick.
                trig1 = nc.gpsimd.trigger_dma(count=None)
                _add_dep(trig1.ins, cc_h.ins, True, "fire gathers after h-AG")
                # 9th prep (w2 chunk 2): ring slot frees once lane0's first
                # frame lands; anchor after trigger1 so the Pool stream
                # cannot stall pre-AG on the flow-control wait.
                late_w2 = gather_window(2, chunks=tuple(range(W2E, NCKS)))
                trig1b = trig1
                if late_w2:
                    for _p in late_w2:
                        _add_dep(_p.ins, trig1.ins, True,
                                 "ring slot after trigger1")
                    trig1b = nc.gpsimd.trigger_dma(count=None)
                # ed[dst] per edge via indT matmuls — runs during the h AllGather
                edp_sb = wk2.tile([128, NW * KMT * GH], F32)
                with tc.tile_pool(name="psE", bufs=1, space="PSUM") as psE:
                    for w in range(NW):
                        edp = psE.tile([128, KMT * GH], F32, tag="edp", bufs=2)
                        for t in range(KMT):
                            nc.tensor.matmul(
                                edp[:, t * GH:(t + 1) * GH],
                                lhsT=indT_sb[:, (w * KMT + t) * 128:
                                             (w * KMT + t + 1) * 128],
                                rhs=edlocB[:, w * GH:(w + 1) * GH],
                                start=True, stop=True,
                            )
                        nc.vector.tensor_copy(
                            edp_sb[:, w * KMT * GH:(w + 1) * KMT * GH], edp[:])
                # self-loop weights for all windows, also during the AG
                wself_all = wk2.tile([128, NW * GH], F32)
                nc.vector.tensor_tensor(out=wself_all[:], in0=esloc[:],
                                        in1=edloc[:], op=mybir.AluOpType.add)
                nc.vector.scalar_tensor_tensor(
                    out=wself_all[:], in0=wself_all[:], scalar=NEG,
                    in1=wself_all[:],
                    op0=mybir.AluOpType.mult, op1=mybir.AluOpType.max)
                nc.scalar.activation(wself_all[:], wself_all[:],
                                     mybir.ActivationFunctionType.Exp)

                # ================= edge phase =================
                CC = 3   # compute-chunk ktiles (gather frames stay CHK)
                NCK = (KMT + CC - 1) // CC
                with tc.tile_pool(name="pe", bufs=1) as pe, \
                     tc.tile_pool(name="psC", bufs=1, space="PSUM") as psC:
                    for w in range(NW):
                        hg = hg_bufs[w % 3]
                        hg3 = hg[:].rearrange("p (c x) -> p c x", x=HROW)
                        ind_w = indF_sb[:, w * KMT * 128:(w + 1) * KMT * 128]
                        wv = pe.tile([128, KMT * GH], F32, tag="wv", bufs=2)
                        ew_ps = psC.tile([128, EW], F32, tag="ew", bufs=2)
                        # w = exp(leaky(es[src] + ed[dst])) batched for the
                        # whole window (3 ops instead of 3 per chunk; the
                        # window's gather frames land well before needed)
                        wvw = wv[:].rearrange("p (c g) -> p c g", g=GH)
                        nc.vector.tensor_tensor(
                            out=wvw,
                            in0=hg3[:, :, G * H * F: G * H * F + GH],
                            in1=edp_sb[:, w * KMT * GH:(w + 1) * KMT * GH]
                            .rearrange("p (c g) -> p c g", g=GH),
                            op=mybir.AluOpType.add,
                        )
                        nc.vector.scalar_tensor_tensor(
                            out=wv[:], in0=wv[:], scalar=NEG, in1=wv[:],
                            op0=mybir.AluOpType.mult, op1=mybir.AluOpType.max,
                        )
                        nc.scalar.activation(
                            wv[:], wv[:], mybir.ActivationFunctionType.Exp)
                        for c0 in range(NCK):
                            t0 = c0 * CC
                            nk = min(CC, KMT - t0)
                            # split the w*h expand-multiply between engines:
                            # gh pairs [0:GS) multiply on DVE at 1x (f32 wv
                            # broadcast); [GS:GH) broadcast-Copy on the
                            # Scalar engine into rhs, then a pure-bf16
                            # packed DVE in-place multiply at 2x_1p.
                            # Balances DVE vs Scalar so neither serializes.
                            GS = 4
                            rhs = pe.tile([128, CC * EW], BF, tag="rhs", bufs=3)
                            rhsv = rhs[:, : nk * EW].rearrange(
                                "p (c x) -> p c x", x=EW)
                            hgv = hg[:, t0 * HROW:(t0 + nk) * HROW].rearrange(
                                "p (c g f) -> p c g f", g=28, f=F)
                            wvb = wv[:, t0 * GH:(t0 + nk) * GH]\
                                .rearrange("p (c g) -> p c g", g=GH)
                            wexpv = rhsv[:, :, GS * F:G * H * F].rearrange(
                                "p c (g f) -> p c g f", f=F)
                            nc.scalar.activation(
                                wexpv,
                                wvb[:, :, GS:GH].to_broadcast([128, nk, GH - GS, F]),
                                mybir.ActivationFunctionType.Copy)
                            nc.vector.tensor_tensor(
                                out=rhsv[:, :, 0:GS * F]
                                .rearrange("p c (g f) -> p c g f", f=F),
                                in0=hgv[:, :, 0:GS, :],
                                in1=wvb[:, :, 0:GS]
                                .to_broadcast([128, nk, GS, F]),
                                op=mybir.AluOpType.mult,
                            )
                            nc.vector.tensor_tensor(
                                out=wexpv,
                                in0=hgv[:, :, GS:GH, :],
                                in1=wexpv,
                                op=mybir.AluOpType.mult,
                            )
                            nc.vector.tensor_copy(
                                rhsv[:, :, G * H * F:EW], wvb)
                            for ck in range(nk):
                                t = t0 + ck
                                first = (t == 0)
                                last = (t == KMT - 1)
                                nc.tensor.matmul(
                                    ew_ps[:, 0:512],
                                    lhsT=ind_w[:, t * 128:(t + 1) * 128],
                                    rhs=rhs[:, ck * EW: ck * EW + 512],
                                    start=first, stop=last, skip_group_check=True,
                                )
                                nc.tensor.matmul(
                                    ew_ps[:, 512:EW],
                                    lhsT=ind_w[:, t * 128:(t + 1) * 128],
                                    rhs=rhs[:, ck * EW + 512:(ck + 1) * EW],
                                    start=first, stop=last, skip_group_check=True,
                                )

                        # late prep+fire for the rotating window-3 buffer
                        # (WAR on hg0 pins desc-gen after w0's readers; the
                        # trig1b dep keeps the SWDGE ring FIFO in emission
                        # order — a w3 frame before w2c2 would be fired by
                        # the wrong trigger)
                        if w == 0:
                            for _p in gather_window(3):
                                _add_dep(_p.ins, trig1b.ins, True,
                                         "ring FIFO: w3 preps after w2c2")
                            nc.gpsimd.trigger_dma(count=None)

                        # ---- self loop + normalize + ELU(+1) for this window
                        mt = w
                        wself = wself_all[:, mt * GH:(mt + 1) * GH]
                        pseH = ew_ps[:, 0:G * H * F].rearrange(
                            "p (g f) -> p g f", f=F)
                        pseZ = ew_ps[:, G * H * F:EW].rearrange(
                            "p (g o) -> p g o", o=1)
                        nc.vector.tensor_tensor(
                            out=pseZ, in0=pseZ,
                            in1=wself.rearrange("p (g o) -> p g o", o=1),
                            op=mybir.AluOpType.add)
                        tmp = pe.tile([128, G * H * F], F32, tag="selfh", bufs=1)
                        nc.vector.tensor_tensor(
                            out=tmp[:].rearrange("p (g f) -> p g f", f=F),
                            in0=hes_sb[:, mt * HROW: mt * HROW + G * H * F]
                            .rearrange("p (g f) -> p g f", f=F),
                            in1=wself.rearrange("p (g o) -> p g o", o=1)
                            .to_broadcast([128, GH, F]),
                            op=mybir.AluOpType.mult)
                        nc.vector.tensor_tensor(
                            out=pseH, in0=pseH,
                            in1=tmp[:].rearrange("p (g f) -> p g f", f=F),
                            op=mybir.AluOpType.add)
                        zv = pe.tile([128, GH], F32, tag="zv", bufs=1)
                        nc.vector.tensor_copy(
                            zv[:].rearrange("p (g o) -> p g o", o=1), pseZ)
                        nc.vector.reciprocal(zv[:], zv[:])
                        t1 = pe.tile([128, G * H * F], F32, tag="t1", bufs=1)
                        nc.vector.tensor_tensor(
                            out=t1[:].rearrange("p (g f) -> p g f", f=F),
                            in0=pseH,
                            in1=zv[:].rearrange("p (g o) -> p g o", o=1)
                            .to_broadcast([128, GH, F]),
                            op=mybir.AluOpType.mult)
                        nc.vector.tensor_add(t1[:], t1[:], bias_sb[:])
                        t2 = pe.tile([128, G * H * F], F32, tag="t2", bufs=1)
                        nc.vector.tensor_scalar_min(t2[:], t1[:], 0.0)
                        nc.scalar.activation(t2[:], t2[:],
                                             mybir.ActivationFunctionType.Exp)
                        elu_w = pe.tile([128, G * H * F], BF, tag="elu", bufs=2)
                        nc.vector.scalar_tensor_tensor(
                            out=elu_w[:], in0=t1[:], scalar=0.0, in1=t2[:],
                            op0=mybir.AluOpType.max, op1=mybir.AluOpType.add)
                        # pair-transpose branches into s1T during the edge phase
                        for g0 in range(0, G, 2):
                            ng = min(2, G - g0)
                            t = g0 // 2
                            p_t3 = psC.tile([128, 128], BF, tag="tp3", bufs=1)
                            nc.tensor.transpose(
                                p_t3[: ng * NHID, :],
                                elu_w[:, g0 * H * F:(g0 + ng) * H * F],
                                ident[:, :],
                            )
                            nc.vector.tensor_copy(
                                s1T[: ng * NHID,
                                    t * S + mt * 128:t * S + (mt + 1) * 128],
                                p_t3[: ng * NHID, :])

                        # ---- per-window MLP + head: fills gather-paced
                        # slack in the edge phase instead of a serial tail.
                        # Block-diagonal pair weights -> one matmul per pair;
                        # ELU(+1) batched across all 7 pairs in 3 wide DVE
                        # ops (the t=6 pad rows yield exactly the constant-1
                        # column the head bias trick needs).
                        s2w = pe.tile([128, 7 * 128], BF, tag="s2w", bufs=2)
                        pm_all = psC.tile([128, 7 * 128], F32, tag="mlp", bufs=1)
                        for t in range(7):
                            nc.tensor.matmul(
                                pm_all[:, t * 128:(t + 1) * 128],
                                lhsT=mw_sb[:, t * 128:(t + 1) * 128],
                                rhs=s1T[:, t * S + mt * 128:
                                        t * S + (mt + 1) * 128],
                                start=True, stop=True, skip_group_check=True)
                        yb = pe.tile([128, 7 * 128], F32, tag="yb", bufs=2)
                        nc.vector.tensor_tensor(
                            out=yb[:].rearrange("p (t c) -> p t c", c=128),
                            in0=pm_all[:].rearrange("p (t c) -> p t c", c=128),
                            in1=mbp2_sb[:].rearrange("p (t o) -> p t o", o=1)
                            .to_broadcast([128, 7, 128]),
                            op=mybir.AluOpType.add)
                        ym = pe.tile([128, 7 * 128], F32, tag="ym", bufs=2)
                        nc.vector.tensor_scalar_min(ym[:], yb[:], 0.0)
                        nc.scalar.activation(ym[:], ym[:],
                                             mybir.ActivationFunctionType.Exp)
                        nc.vector.scalar_tensor_tensor(
                            out=s2w[:], in0=yb[:], scalar=0.0, in1=ym[:],
                            op0=mybir.AluOpType.max, op1=mybir.AluOpType.add)
                        p_fw = psC.tile([C, 128], F32, tag="fin", bufs=1)
                        for t in range(7):
                            nc.tensor.matmul(
                                p_fw[:], lhsT=outw_sb[:, t * C:(t + 1) * C],
                                rhs=s2w[:, t * 128:(t + 1) * 128],
                                start=(t == 0), stop=(t == 6))
                        nc.vector.tensor_copy(lg[:, mt * 128:(mt + 1) * 128],
                                              p_fw[:])

                # ================= log_softmax =================
                with tc.tile_pool(name="pf", bufs=1) as pf, \
                     tc.tile_pool(name="psD", bufs=1, space="PSUM") as psD:
                    # batched log_softmax over all 4 node windows
                    lt = pf.tile([128, NW * C], F32)
                    for mt in range(NW):
                        p_l = psD.tile([128, C], F32, tag="lsm", bufs=2)
                        nc.tensor.transpose(p_l[:], lg[:, mt * 128:(mt + 1) * 128],
                                            identf[:C, :C])
                        nc.vector.tensor_copy(lt[:, mt * C:(mt + 1) * C], p_l[:])
                    lt3 = lt[:].rearrange("p (m c) -> p m c", c=C)
                    mx = pf.tile([128, NW], F32)
                    nc.vector.reduce_max(mx[:].rearrange("p (m o) -> p m o", o=1),
                                         lt3, axis=mybir.AxisListType.X)
                    nc.vector.tensor_tensor(
                        out=lt3, in0=lt3,
                        in1=mx[:].rearrange("p (m o) -> p m o", o=1)
                        .to_broadcast([128, NW, C]),
                        op=mybir.AluOpType.subtract)
                    ex = pf.tile([128, NW * C], F32)
                    nc.scalar.activation(ex[:], lt[:], mybir.ActivationFunctionType.Exp)
                    se = pf.tile([128, NW], F32)
                    nc.vector.reduce_sum(se[:].rearrange("p (m o) -> p m o", o=1),
                                         ex[:].rearrange("p (m c) -> p m c", c=C),
                                         axis=mybir.AxisListType.X)
                    nc.scalar.activation(se[:], se[:], mybir.ActivationFunctionType.Ln)
                    oo = pf.tile([128, NW * C], F32)
                    nc.vector.tensor_tensor(
                        out=oo[:].rearrange("p (m c) -> p m c", c=C),
                        in0=lt3,
                        in1=se[:].rearrange("p (m o) -> p m o", o=1)
                        .to_broadcast([128, NW, C]),
                        op=mybir.AluOpType.subtract)
                    nc.sync.dma_start(
                        d_out[:].rearrange("(m p) c -> p m c", p=128),
                        oo[:].rearrange("p (m c) -> p m c", c=C))

    # re-executability: zero the gather-completion sem after Tile's teardown
    # drains (which, post-patch, wait for all prepared gathers to land)
    nc.clear_and_free_semaphores(hg_sems)
    _patch_prep_waits(nc)
    nc.compile()
    return nc


def _patch_prep_waits(nc):
    """Anything gating on a prepared gather (data consumers, SWDGE ring
    flow-control IncSwdgeSem, end-of-kernel drains) waits on Tile's DMASW
    lane sems, which nothing increments in the prepare_only flow — the DMA
    completion sem is the descriptor-baked per-lane hg_dma{L} instead.
    Retarget every DMASW{L} wait to hg_dma{L}, keeping the tick value
    (lane-local frame count * 16). Lane assignment is round-robin over the
    8 DMASW procs in scheduled order; assert it matches emission order
    (prep j -> lane j%8) via each prep's baked sem."""
    import re
    preps = [ins for ins in nc.inst_map.values()
             if type(ins).__name__ == "InstDMAGatherAnt"
             and getattr(ins, "gen_mode", 0) == 1]
    assert preps, "no prepared gathers found"
    lane_sems = {}
    for j, p in enumerate(preps):
        upd = p.sync_info.on_update[0]
        assert upd.ant_name == f"hg_dma{j % 8}", (j, upd.ant_name)
        lane_sems[j % 8] = (upd.id, upd.ant_name)
    # lane round-robin follows SCHEDULED order; the baked sems follow
    # EMISSION order — verify they coincide by walking the pool stream
    sched = []
    for b in nc.main_func.blocks:
        for ins in b.instructions:
            if (type(ins).__name__ == "InstDMAGatherAnt"
                    and getattr(ins, "gen_mode", 0) == 1):
                sched.append(ins.name)
    assert sched == [p.name for p in preps], (
        f"prep scheduled order != emission order:\n{sched}\n"
        f"{[p.name for p in preps]}")
    for ins in nc.inst_map.values():
        si = getattr(ins, "sync_info", None)
        if not si or not si.on_wait:
            continue
        for w in si.on_wait:
            m = re.match(r"DMASW(\d+)_", w.ant_name or "")
            if m and w.wait_mode == "sem-ge-imm":
                lane = int(m.group(1))
                assert lane in lane_sems, (lane, w.wait_value)
                sid, sname = lane_sems[lane]
                w.id = sid
                w.ant_name = sname


def _host_prep(inputs):
    """Shard/transpose/cast inputs; build edge structures."""
    x = np.asarray(inputs["x"], np.float32)
    edge_index = np.asarray(inputs["edge_index"]).astype(np.int64)
    U = np.asarray(inputs["U"], np.float32)
    psi = np.asarray(inputs["psi"], np.float32)
    gat_W = np.asarray(inputs["gat_W"], np.float32)
    att_src = np.asarray(inputs["att_src"], np.float32)
    att_dst = np.asarray(inputs["att_dst"], np.float32)
    gat_b = np.asarray(inputs["gat_b"], np.float32)
    mlp_W = np.asarray(inputs["mlp_W"], np.float32)
    mlp_b = np.asarray(inputs["mlp_b"], np.float32)
    out_W = np.asarray(inputs["out_W"], np.float32)
    out_b = np.asarray(inputs["out_b"], np.float32)

    src, dst = edge_index[0], edge_index[1]

    core_all = dst // S
    win_all = (dst % S) // 128
    key = core_all * NW + win_all
    order = np.argsort(key, kind="stable")
    counts = np.bincount(key, minlength=R * NW)
    maxw = counts.max()
    KMT = int((maxw + 127) // 128)
    KE = NW * KMT * 128
    TE = NW * KMT

    # shared weight packs
    wcat = np.zeros((F, G * 68), np.float32)
    for g in range(G):
        Wg = gat_W[g]                                   # [F, H*F]
        Wh = Wg.reshape(F, H, F)
        Ws = np.einsum("ihf,hf->ih", Wh, att_src[g])    # [F, H]
        Wd = np.einsum("ihf,hf->ih", Wh, att_dst[g])    # [F, H]
        wcat[:, g * 68: g * 68 + H * F] = Wg
        wcat[:, g * 68 + H * F: g * 68 + H * F + H] = Ws
        wcat[:, g * 68 + H * F + H: g * 68 + 68] = Wd
    bias = np.tile(gat_b.reshape(1, G * H * F), (128, 1)).astype(np.float32)
    # block-diagonal pair-packed MLP weights: one [128,128] matmul per pair
    mw = np.zeros((128, 7 * 128), np.float32)
    for t in range(7):
        for gg in range(min(2, G - 2 * t)):
            mw[gg * NHID:(gg + 1) * NHID,
               t * 128 + gg * NHID:t * 128 + (gg + 1) * NHID] = mlp_W[2 * t + gg]
    mbp = np.stack([mlp_b[g] - mlp_W[g].sum(0) for g in range(G)], 1)
    mbp2 = np.zeros((128, 7), np.float32)
    for g in range(G):
        mbp2[(g % 2) * NHID:(g % 2 + 1) * NHID, g // 2] = mbp[:, g]
    outw = np.zeros((7 * 128, C), np.float32)
    outw[:G * NHID, :] = out_W
    outw[G * NHID, :] = out_b - out_W.sum(0)
    outwW = _wrap128(outw)                              # [128, 7*C]

    af = np.abs(x)
    afW = _wrap128(af)                                  # [128, KT*F]

    def wrap_idx(arr):
        a = arr.reshape(-1, 16).T.astype(np.int16)
        return np.ascontiguousarray(np.tile(a, (8, 1)))

    starts = np.zeros(R * NW + 1, np.int64)
    starts[1:] = np.cumsum(counts)
    sorted_e = order

    in_maps = []
    for r in range(R):
        sl = slice(r * S, (r + 1) * S)
        # [m, kt-major] layouts: psiW[p, kt*J*S + j*S + n] = psi[j, r*S+n, kt*128+p]
        psiT = np.ascontiguousarray(
            psi[:, sl, :].transpose(2, 0, 1).reshape(N, J * S))
        psiW = _wrap128(psiT)
        uT = np.ascontiguousarray(U[sl, :].T)           # [N, S]
        uW = _wrap128(uT)

        gsrc = np.zeros(KE, np.int64)
        ldst = np.zeros(KE, np.int64)
        valid = np.zeros(KE, bool)
        for w in range(NW):
            k = r * NW + w
            es = sorted_e[starts[k]:starts[k + 1]]
            base = w * KMT * 128
            gsrc[base: base + len(es)] = src[es]
            ldst[base: base + len(es)] = dst[es] - r * S
            valid[base: base + len(es)] = True
        # window-relative dst (0..127), -1 for padding
        w_of = (np.arange(KE) // 128) // KMT
        rel = ldst - 128 * w_of
        tt = np.arange(KE) // 128
        pp = np.arange(KE) % 128
        relv = rel[valid].astype(np.int64)
        indT = np.zeros((128, TE * 128), np.float32)
        cols = tt * 128 + pp
        indT[relv, cols[valid]] = 1.0
        indF = np.zeros((128, TE * 128), np.float32)
        indF[pp[valid], tt[valid] * 128 + relv] = 1.0

        in_maps.append({
            "af": _bf(afW),
            "psiW": _bf(psiW),
            "uW": _bf(uW),
            "wcat": _bf(wcat),
            "bias": bias,
            "mw": _bf(mw),
            "mbp2": _f32(mbp2),
            "outwW": _bf(outwW),
            "gidx": wrap_idx(gsrc),
            "indF": _bf(indF),
            "indT": _bf(indT),
        })
    return in_maps, KMT, CHK


def kernel(**inputs) -> np.ndarray:
    in_maps, KMT, _ = _host_prep(inputs)
    if KMT not in _PROGRAM_CACHE:
        _PROGRAM_CACHE[KMT] = build_program(KMT)
    nc = _PROGRAM_CACHE[KMT]
    res = run_bass_kernel_spmd(nc, in_maps, list(range(R)))
    out = np.concatenate([res.results[i]["out"] for i in range(R)], axis=0)
    return out.astype(np.float32)



# revision 54
# speedup vs baseline: 1.0569x; 1.0569x over previous
"""Trainium2 Bass kernel for the 13-branch scattering-GAT network.

Strategy (8 NeuronCores, row-parallel, v3):
  - Nodes sharded 512/core. All constant inputs host-prewrapped to
    partition-major [128, X] contiguous layouts so every load is a fat DMA.
  - psi streamed per 2-ktile chunk on alternating sync/scalar DMA queues
    (read twice: level-1 and level-2 wavelets), freeing SBUF for gather
    buffers. A tiny warmup AllGather absorbs the CC first-trigger latency.
  - Three AllGathers (|y1|, |y2|, h||es) write Shared-scratchpad DRAM.
  - Edge softmax-aggregation: per-edge rows of the AllGathered h table are
    fetched with prepare_only dma_gather. Descriptors for 8 frames (SWDGE
    ring depth: 1 per lane x 8 lanes) are generated on GpSimd during the
    wavelet phases and fired by one trigger_dma right after the h-AG
    (explicit sync dep; the gathers read an address alias so Tile cannot
    invert the dependency). Remaining frames prep behind trigger1/w0
    readers. Per-lane hg_dma sems replace Tile's un-incremented DMASW
    waits via _patch_prep_waits.
  - ed[dst] lookup per edge is a small PE matmul against a host-shipped
    transposed 0/1 indicator (runs inside the h-AG window). Aggregation is
    dense 128-edge-tile matmuls into PSUM; the exp(leaky) edge-weight
    expand-multiply is split DVE-1x / Scalar-expand + DVE-2x (bf16 packed).
  - Self-loops folded in locally; per-window MLP + head run inside the
    edge loop (fills gather-paced slack); batched log_softmax at the end.
"""

import sys

sys.path.insert(0, "/opt/trn_rl_repo")

import numpy as np
import ml_dtypes

import concourse.bass as bass
import concourse.mybir as mybir
import concourse.tile as tile
from concourse import bacc
from concourse.bass import _add_dep_helper
from concourse.bass_utils import run_bass_kernel_spmd


def _add_dep(dependent, dependency, sync, reason):
    _add_dep_helper(dependent, dependency, sync=sync, reason=reason)

R = 8          # cores
N = 4096       # nodes
S = N // R     # nodes per core (512)
F = 32         # features
H = 2          # heads
G = 13         # branches
GH = G * H     # 26
NHID = 64
C = 10
J = 3
KT = N // 128  # 32 contraction tiles
NW = S // 128  # 4 dst windows per core
HROW = 896     # padded AG row width (1792B, 256B-aligned)
EW = GH * 33   # 858 edge-matmul output width per dst window
NEG = 0.2
CHK = 6        # ktiles per rhs-build chunk

BF = mybir.dt.bfloat16
F32 = mybir.dt.float32
I16 = mybir.dt.int16

_bf = lambda a: np.ascontiguousarray(a.astype(ml_dtypes.bfloat16))
_f32 = lambda a: np.ascontiguousarray(a.astype(np.float32))

_PROGRAM_CACHE = {}


def _wrap128(a):
    """[KT*128, X] -> [128, KT*X] partition-major."""
    n, x = a.shape
    k = n // 128
    return np.ascontiguousarray(a.reshape(k, 128, x).transpose(1, 0, 2).reshape(128, k * x))


def build_program(KMT):
    TE = NW * KMT            # total edge k-tiles
    KE = TE * 128            # padded edge count
    nc = bacc.Bacc("TRN2", target_bir_lowering=False, debug=False, num_devices=R)

    # ---------------- I/O (all host-prewrapped partition-major) -------------
    d_af = nc.dram_tensor("af", [128, KT * F], BF, kind="ExternalInput")
    d_psi = nc.dram_tensor("psiW", [128, KT * J * S], BF, kind="ExternalInput")
    d_u = nc.dram_tensor("uW", [128, KT * S], BF, kind="ExternalInput")
    d_wcat = nc.dram_tensor("wcat", [F, G * 68], BF, kind="ExternalInput")
    d_bias = nc.dram_tensor("bias", [128, G * H * F], F32, kind="ExternalInput")
    d_mw = nc.dram_tensor("mw", [128, 7 * 128], BF, kind="ExternalInput")
    d_mbp = nc.dram_tensor("mbp2", [128, 7], F32, kind="ExternalInput")
    d_outw = nc.dram_tensor("outwW", [128, 7 * C], BF, kind="ExternalInput")
    d_gidx = nc.dram_tensor("gidx", [128, KE // 16], I16, kind="ExternalInput")
    d_indF = nc.dram_tensor("indF", [128, TE * 128], BF, kind="ExternalInput")
    d_indT = nc.dram_tensor("indT", [128, TE * 128], BF, kind="ExternalInput")
    d_out = nc.dram_tensor("out", [S, C], F32, kind="ExternalOutput")

    from concourse.masks import make_identity

    with tile.TileContext(nc) as tc:
        with (
            tc.tile_pool(name="const", bufs=1) as kc,
            tc.tile_pool(name="work", bufs=1) as wk,
        ):
            # ---------------- constant loads ----------------
            af_sb = kc.tile([128, KT * F], BF)
            nc.sync.dma_start(af_sb[:], d_af[:])
            gidx_sb = kc.tile([128, KE // 16], I16)
            nc.scalar.dma_start(gidx_sb[:], d_gidx[:])
            wcat_sb = kc.tile([F, G * 68], BF)
            nc.scalar.dma_start(wcat_sb[:], d_wcat[:])
            bias_sb = kc.tile([128, G * H * F], F32)
            nc.scalar.dma_start(bias_sb[:], d_bias[:])
            mw_sb = kc.tile([128, 7 * 128], BF)
            nc.scalar.dma_start(mw_sb[:], d_mw[:])
            mbp2_sb = kc.tile([128, 7], F32)
            nc.scalar.dma_start(mbp2_sb[:], d_mbp[:])
            outw_sb = kc.tile([128, 7 * C], BF)
            nc.scalar.dma_start(outw_sb[:], d_outw[:])

            ident = kc.tile([128, 128], BF)
            make_identity(nc, ident[:])
            identf = kc.tile([128, 128], F32)
            make_identity(nc, identf[:])

            # tiny warmup AllGather issued at t~0: absorbs the CC-stack
            # first-trigger latency (~11us) so AG-a1 triggers fast
            warm_in = nc.dram_tensor("warm_in", [1, 64], BF, kind="Internal")
            warm_out = nc.dram_tensor("warm_out", [R, 64], BF, kind="Internal",
                                      addr_space="Shared")
            nc.gpsimd.collective_compute(
                "AllGather", mybir.AluOpType.bypass,
                replica_groups=[list(range(R))],
                ins=[warm_in[:].opt()], outs=[warm_out[:].opt()],
            )

            # ---------------- persistent work tiles ----------------
            indT_sb = wk.tile([128, TE * 128], BF)
            indF_sb = wk.tile([128, TE * 128], BF)
            hes_sb = wk.tile([128, NW * HROW], BF)
            nc.vector.memset(
                hes_sb[:].rearrange("p (m c) -> p m c", c=HROW)[:, :, EW:HROW], 0.0)
            esloc = wk.tile([128, NW * GH], F32)
            edloc = wk.tile([128, NW * GH], F32)
            edlocB = wk.tile([128, NW * GH], BF)
            # 3 rotating whole-window gather buffers
            hg_bufs = [wk.tile([128, KMT * HROW], BF, tag="hg", bufs=3,
                               name=f"hg{i}") for i in range(3)]

            # DRAM: AG staging (Local in, Shared out)
            aga1_in = nc.dram_tensor("aga1_in", [S, J * F], BF, kind="Internal")
            aga1_out = nc.dram_tensor("aga1_out", [N, J * F], BF, kind="Internal")
            aga2_in = nc.dram_tensor("aga2_in", [S, J * J * F], BF, kind="Internal")
            aga2_out = nc.dram_tensor("aga2_out", [N, J * J * F], BF, kind="Internal", addr_space="Shared")
            agh_in = nc.dram_tensor("agh_in", [S, HROW], BF, kind="Internal")
            agh_out = nc.dram_tensor("agh_out", [N, HROW], BF, kind="Internal", addr_space="Shared")
            # alias of agh_out for the prepared gathers: hides the read from
            # Tile's dep tracker (else the AG inherits a WAR wait on gather
            # DMAs that only fire post-AG -> deadlock). Ordering is restored
            # by an explicit sync dep from trigger_dma onto the h-AG
            # instruction (Tile emits the Collectives-tick wait from it).
            agh_rd = nc.dram_tensor("agh_rd", [N, HROW], BF, kind="Internal", addr_space="Shared")
            nc.lookup_mloc(agh_rd).addr = nc.lookup_mloc(agh_out).addr
            rg = [list(range(R))]
            # one DMA-completion sem per SWDGE lane (8 lanes, round-robin in
            # scheduled order = emission order); exact lane-FIFO semantics
            hg_sems = [nc.alloc_semaphore(f"hg_dma{i}") for i in range(8)]
            _prep_ctr = [0]

            def gather_window(w, chunks=None):
                # prepare_only: descriptors generated on GpSimd NOW (off the
                # post-AG critical path); the DMA fires at the next
                # trigger_dma. chunked <=768 idxs per call (ISA limit).
                # SWDGE ring holds 1 outstanding frame per lane (8 total):
                # at most 8 preps may be pending before the first trigger.
                hg = hg_bufs[w % 3]
                ncks = (KMT + CHK - 1) // CHK
                out = []
                for c in range(ncks) if chunks is None else chunks:
                    t0 = c * CHK
                    nk = min(CHK, KMT - t0)
                    j = _prep_ctr[0]
                    _prep_ctr[0] += 1
                    out.append(nc.gpsimd.dma_gather(
                        out_ap=hg[:, t0 * HROW:(t0 + nk) * HROW]
                        .rearrange("p (c x) -> p c x", x=HROW),
                        in_ap=agh_rd[:],
                        idxs_ap=gidx_sb[:, (w * KMT + t0) * 8:
                                        (w * KMT + t0 + nk) * 8],
                        num_idxs=nk * 128,
                        num_idxs_reg=nk * 128,
                        elem_size=HROW,
                        prepare_only=True,
                        sem=hg_sems[j % 8],
                    ))
                return out

            # window-0 desc-gen early (GpSimd idle during phase 2)
            gather_window(0)


            # ============ phases 2-5: wavelet tree + coefs + GAT linear ======
            NPC = 2  # ktiles per psi chunk
            with tc.tile_pool(name="st", bufs=1) as st:
                a1_sb = st.tile([128, KT * J * F], BF)
                coefsT2 = st.tile([F, G * S], BF)
                a1T = st.tile([F, J * S], BF)
                a1loc = st.tile([128, NW * J * F], BF)
                a2T = st.tile([J * F, J * S], BF)
                a2loc = st.tile([128, NW * J * J * F], BF)

                # ---- phase 2: y1_j = psi_j @ af (psi streamed)
                with tc.tile_pool(name="psA2", bufs=1, space="PSUM") as psA2:
                    p_y1 = [psA2.tile([F, S], F32, tag=f"y1{j}", bufs=1,
                                      name=f"y1{j}") for j in range(J)]
                    for kc_ in range(KT // NPC):
                        psi_c = st.tile([128, NPC * J * S], BF, tag="psi", bufs=4)
                        (nc.sync if kc_ % 2 == 0 else nc.scalar).dma_start(
                            psi_c[:],
                            d_psi[:, kc_ * NPC * J * S:(kc_ + 1) * NPC * J * S])
                        for dk in range(NPC):
                            kt = kc_ * NPC + dk
                            for j in range(J):
                                nc.tensor.matmul(
                                    p_y1[j][:],
                                    lhsT=af_sb[:, kt * F:(kt + 1) * F],
                                    rhs=psi_c[:, dk * J * S + j * S:
                                              dk * J * S + (j + 1) * S],
                                    start=(kt == 0), stop=(kt == KT - 1),
                                )
                    for j in range(J):
                        nc.scalar.activation(
                            a1T[:, j * S:(j + 1) * S], p_y1[j][:],
                            mybir.ActivationFunctionType.Abs)
                    for j in range(J):
                        for mt in range(NW):
                            p_tp = psA2.tile([128, F], BF, tag="tp", bufs=2)
                            nc.tensor.transpose(
                                p_tp[:],
                                a1T[:, j * S + mt * 128: j * S + (mt + 1) * 128],
                                ident[:F, :F],
                            )
                            nc.vector.tensor_copy(
                                a1loc[:, mt * J * F + j * F: mt * J * F + (j + 1) * F],
                                p_tp[:],
                            )
                for mt in range(NW):
                    nc.sync.dma_start(
                        aga1_in[mt * 128:(mt + 1) * 128, :],
                        a1loc[:, mt * J * F:(mt + 1) * J * F],
                    )
                nc.gpsimd.collective_compute(
                    "AllGather", mybir.AluOpType.bypass, replica_groups=rg,
                    ins=[aga1_in[:].opt()], outs=[aga1_out[:].opt()],
                )
                # windows 1-2 desc-gen during the a1 AllGather / phase 3
                # (ring cap: at most 8 preps pending before trigger1)
                NCKS = (KMT + CHK - 1) // CHK
                W2E = max(0, min(NCKS, 8 - 2 * NCKS))
                gather_window(1)
                gather_window(2, chunks=tuple(range(W2E)))
                nc.scalar.dma_start(indT_sb[:], d_indT[:])
                nc.scalar.dma_start(
                    a1_sb[:].rearrange("p (k c) -> p k c", c=J * F),
                    aga1_out[:].rearrange("(k p) c -> p k c", p=128),
                )

                # ---- phase 4a: coef g=0 (af group) — runs during AG-a1
                with tc.tile_pool(name="psB", bufs=1, space="PSUM") as psB:
                    pA = psB.tile([F, S], F32, tag="cA", bufs=1, name="cA")
                    for kt4 in range(KT // 4):
                        u_c = st.tile([128, 4 * S], BF, tag="u", bufs=2)
                        nc.scalar.dma_start(
                            u_c[:], d_u[:, kt4 * 4 * S:(kt4 + 1) * 4 * S])
                        for dk in range(4):
                            kt = kt4 * 4 + dk
                            nc.tensor.matmul(
                                pA[:], lhsT=af_sb[:, kt * F:(kt + 1) * F],
                                rhs=u_c[:, dk * S:(dk + 1) * S],
                                start=(kt == 0), stop=(kt == KT - 1))
                    nc.vector.tensor_copy(coefsT2[:, 0:S], pA[:])

                # ---- phase 3: y2_{j,k} = psi_k @ a1_j (psi streamed again)
                with tc.tile_pool(name="psA3", bufs=1, space="PSUM") as psA3:
                    p_y2 = [psA3.tile([J * F, S], F32, tag=f"y2{k}", bufs=1,
                                      name=f"y2{k}") for k in range(J)]
                    for kc_ in range(KT // NPC):
                        psi_c = st.tile([128, NPC * J * S], BF, tag="psi", bufs=4)
                        (nc.sync if kc_ % 2 == 0 else nc.scalar).dma_start(
                            psi_c[:],
                            d_psi[:, kc_ * NPC * J * S:(kc_ + 1) * NPC * J * S])
                        for dk in range(NPC):
                            kt = kc_ * NPC + dk
                            for k in range(J):
                                nc.tensor.matmul(
                                    p_y2[k][:],
                                    lhsT=a1_sb[:, kt * J * F:(kt + 1) * J * F],
                                    rhs=psi_c[:, dk * J * S + k * S:
                                              dk * J * S + (k + 1) * S],
                                    start=(kt == 0), stop=(kt == KT - 1),
                                )
                    for k in range(J):
                        nc.scalar.activation(
                            a2T[:, k * S:(k + 1) * S], p_y2[k][:],
                            mybir.ActivationFunctionType.Abs)
                    for k in range(J):
                        for mt in range(NW):
                            p_tp2 = psA3.tile([128, J * F], BF, tag="tp2", bufs=2)
                            nc.tensor.transpose(
                                p_tp2[:],
                                a2T[:, k * S + mt * 128: k * S + (mt + 1) * 128],
                                ident[:J * F, :J * F],
                            )
                            nc.vector.tensor_copy(
                                a2loc[:, mt * J * J * F + k * J * F:
                                      mt * J * J * F + (k + 1) * J * F],
                                p_tp2[:],
                            )
                for mt in range(NW):
                    nc.sync.dma_start(
                        aga2_in[mt * 128:(mt + 1) * 128, :],
                        a2loc[:, mt * J * J * F:(mt + 1) * J * J * F],
                    )
                nc.gpsimd.collective_compute(
                    "AllGather", mybir.AluOpType.bypass, replica_groups=rg,
                    ins=[aga2_in[:].opt()], outs=[aga2_out[:].opt()],
                )

                # ---- phase 4a2: coef g=1..3 (a1 group) — runs during AG-a2
                with tc.tile_pool(name="psB1b", bufs=1, space="PSUM") as psB1b:
                    pB = psB1b.tile([J * F, S], F32, tag="cB", bufs=1, name="cB")
                    for kt4 in range(KT // 4):
                        u_c = st.tile([128, 4 * S], BF, tag="u", bufs=2)
                        nc.scalar.dma_start(
                            u_c[:], d_u[:, kt4 * 4 * S:(kt4 + 1) * 4 * S])
                        for dk in range(4):
                            kt = kt4 * 4 + dk
                            nc.tensor.matmul(
                                pB[:], lhsT=a1_sb[:, kt * J * F:(kt + 1) * J * F],
                                rhs=u_c[:, dk * S:(dk + 1) * S],
                                start=(kt == 0), stop=(kt == KT - 1))
                    for j in range(J):
                        nc.vector.tensor_copy(
                            coefsT2[:, (1 + j) * S:(2 + j) * S],
                            pB[j * F:(j + 1) * F, :])

                # ---- phase 4b: coef g=4..12 (a2 groups), u + a2 streamed
                with tc.tile_pool(name="psB2", bufs=1, space="PSUM") as psB2:
                    pC = psB2.tile([128, S], F32, tag="cC", bufs=1, name="cC")
                    pD = psB2.tile([128, S], F32, tag="cD", bufs=1, name="cD")
                    pE = psB2.tile([F, S], F32, tag="cE", bufs=1, name="cE")
                    for kt4 in range(KT // 4):
                        u_c = st.tile([128, 4 * S], BF, tag="u", bufs=2)
                        nc.scalar.dma_start(
                            u_c[:], d_u[:, kt4 * 4 * S:(kt4 + 1) * 4 * S])
                        a2_c = st.tile([128, 4 * J * J * F], BF, tag="a2c", bufs=2)
                        nc.sync.dma_start(
                            a2_c[:].rearrange("p (k c) -> p k c", c=J * J * F),
                            aga2_out[kt4 * 512:(kt4 + 1) * 512, :]
                            .rearrange("(k p) c -> p k c", p=128),
                        )
                        for dk in range(4):
                            kt = kt4 * 4 + dk
                            base = dk * J * J * F
                            nc.tensor.matmul(
                                pC[:], lhsT=a2_c[:, base:base + 4 * F],
                                rhs=u_c[:, dk * S:(dk + 1) * S],
                                start=(kt == 0), stop=(kt == KT - 1))
                            nc.tensor.matmul(
                                pD[:], lhsT=a2_c[:, base + 4 * F:base + 8 * F],
                                rhs=u_c[:, dk * S:(dk + 1) * S],
                                start=(kt == 0), stop=(kt == KT - 1))
                            nc.tensor.matmul(
                                pE[:], lhsT=a2_c[:, base + 8 * F:base + 9 * F],
                                rhs=u_c[:, dk * S:(dk + 1) * S],
                                start=(kt == 0), stop=(kt == KT - 1))
                    for ci in range(9):
                        j, k = ci % 3, ci // 3
                        g = 4 + 3 * j + k
                        srcp = (pC, pD, pE)[ci // 4]
                        off = (ci % 4) * F if ci < 8 else 0
                        nc.vector.tensor_copy(
                            coefsT2[:, g * S:(g + 1) * S], srcp[off:off + F, :])

                # ============ phase 5: h||es||ed = coefs @ wcat =============
                with tc.tile_pool(name="psB5", bufs=1, space="PSUM") as psB5:
                    for mt in range(NW):
                        p_h = psB5.tile([128, 1024], F32, tag="h5", bufs=2)
                        for g in range(G):
                            pc = g * 68 if g < 7 else 512 + (g - 7) * 68
                            nc.tensor.matmul(
                                p_h[:, pc:pc + 68],
                                lhsT=coefsT2[:, g * S + mt * 128: g * S + (mt + 1) * 128],
                                rhs=wcat_sb[:, g * 68:(g + 1) * 68],
                                start=True, stop=True,
                            )
                        for g0, ng in ((0, 7), (7, 6)):
                            base = 0 if g0 == 0 else 512
                            ph = p_h[:, base: base + ng * 68]\
                                .rearrange("p (g c) -> p g c", c=68)
                            nc.vector.tensor_copy(
                                hes_sb[:, mt * HROW + g0 * H * F:
                                       mt * HROW + (g0 + ng) * H * F]
                                .rearrange("p (g c) -> p g c", c=H * F),
                                ph[:, :, 0:H * F],
                            )
                            nc.vector.tensor_copy(
                                hes_sb[:, mt * HROW + G * H * F + g0 * H:
                                       mt * HROW + G * H * F + (g0 + ng) * H]
                                .rearrange("p (g c) -> p g c", c=H),
                                ph[:, :, H * F:H * F + H],
                            )
                            nc.vector.tensor_copy(
                                esloc[:, mt * GH + g0 * H: mt * GH + (g0 + ng) * H]
                                .rearrange("p (g c) -> p g c", c=H),
                                ph[:, :, H * F:H * F + H],
                            )
                            nc.vector.tensor_copy(
                                edloc[:, mt * GH + g0 * H: mt * GH + (g0 + ng) * H]
                                .rearrange("p (g c) -> p g c", c=H),
                                ph[:, :, H * F + H:H * F + 2 * H],
                            )
                            nc.vector.tensor_copy(
                                edlocB[:, mt * GH + g0 * H: mt * GH + (g0 + ng) * H]
                                .rearrange("p (g c) -> p g c", c=H),
                                ph[:, :, H * F + H:H * F + 2 * H],
                            )
                        nc.sync.dma_start(
                            agh_in[mt * 128:(mt + 1) * 128, :],
                            hes_sb[:, mt * HROW:(mt + 1) * HROW],
                        )
            # s1T outlives the edge phase (built per-window inside it)
            with tc.tile_pool(name="wk2", bufs=1) as wk2:
                s1T = wk2.tile([128, 7 * S], BF)
                nc.vector.memset(s1T[64:128, 6 * S:7 * S], 0.0)
                lg = wk2.tile([C, S], F32)
                # indF load lands in the h-AG window (sync queue idle);
                # indT loads earlier (after AG-a1) for the edp matmuls
                nc.sync.dma_start(indF_sb[:], d_indF[:])
                cc_h = nc.gpsimd.collective_compute(
                    "AllGather", mybir.AluOpType.bypass, replica_groups=rg,
                    ins=[agh_in[:].opt()], outs=[agh_out[:].opt()],
                )
                # fire the 8 prepared gathers (w0, w1, w2c0-1). The preps
                # read agh_rd (untracked alias), so the h-AG ordering is
                # restored with an explicit sync dep: Tile schedules the
                # trigger after the AG and waits its Collectives tick.
                trig1 = nc.gpsimd.trigger_dma(count=None)
                _add_dep(trig1.ins, cc_h.ins, True, "fire gathers after h-AG")
                # 9th prep (w2 chunk 2): ring slot frees once lane0's first
                # frame lands; anchor after trigger1 so the Pool stream
                # cannot stall pre-AG on the flow-control wait.
                late_w2 = gather_window(2, chunks=tuple(range(W2E, NCKS)))
                trig1b = trig1
                if late_w2:
                    for _p in late_w2:
                        _add_dep(_p.ins, trig1.ins, True,
                                 "ring slot after trigger1")
                    trig1b = nc.gpsimd.trigger_dma(count=None)
                # ed[dst] per edge via indT matmuls — runs during the h AllGather
                edp_sb = wk2.tile([128, NW * KMT * GH], F32)
                with tc.tile_pool(name="psE", bufs=1, space="PSUM") as psE:
                    for w in range(NW):
                        edp = psE.tile([128, KMT * GH], F32, tag="edp", bufs=2)
                        for t in range(KMT):
                            nc.tensor.matmul(
                                edp[:, t * GH:(t + 1) * GH],
                                lhsT=indT_sb[:, (w * KMT + t) * 128:
                                             (w * KMT + t + 1) * 128],
                                rhs=edlocB[:, w * GH:(w + 1) * GH],
                                start=True, stop=True,
                            )
                        nc.vector.tensor_copy(
                            edp_sb[:, w * KMT * GH:(w + 1) * KMT * GH], edp[:])
                # self-loop weights for all windows, also during the AG
                wself_all = wk2.tile([128, NW * GH], F32)
                nc.vector.tensor_tensor(out=wself_all[:], in0=esloc[:],
                                        in1=edloc[:], op=mybir.AluOpType.add)
                nc.vector.scalar_tensor_tensor(
                    out=wself_all[:], in0=wself_all[:], scalar=NEG,
                    in1=wself_all[:],
                    op0=mybir.AluOpType.mult, op1=mybir.AluOpType.max)
                nc.scalar.activation(wself_all[:], wself_all[:],
                                     mybir.ActivationFunctionType.Exp)

                # ================= edge phase =================
                CC = 3   # compute-chunk ktiles (gather frames stay CHK)
                NCK = (KMT + CC - 1) // CC
                with tc.tile_pool(name="pe", bufs=1) as pe, \
                     tc.tile_pool(name="psC", bufs=1, space="PSUM") as psC:
                    for w in range(NW):
                        hg = hg_bufs[w % 3]
                        hg3 = hg[:].rearrange("p (c x) -> p c x", x=HROW)
                        ind_w = indF_sb[:, w * KMT * 128:(w + 1) * KMT * 128]
                        wv = pe.tile([128, KMT * GH], F32, tag="wv", bufs=2)
                        ew_ps = psC.tile([128, EW], F32, tag="ew", bufs=2)
                        for c0 in range(NCK):
                            t0 = c0 * CC
                            nk = min(CC, KMT - t0)
                            # w = exp(leaky(es[src] + ed[dst])), per gather
                            # chunk so compute starts as soon as data lands
                            wvc = wv[:, t0 * GH:(t0 + nk) * GH]\
                                .rearrange("p (c g) -> p c g", g=GH)
                            nc.vector.tensor_tensor(
                                out=wvc,
                                in0=hg3[:, t0:t0 + nk, G * H * F: G * H * F + GH],
                                in1=edp_sb[:, (w * KMT + t0) * GH:
                                           (w * KMT + t0 + nk) * GH]
                                .rearrange("p (c g) -> p c g", g=GH),
                                op=mybir.AluOpType.add,
                            )
                            nc.vector.scalar_tensor_tensor(
                                out=wvc, in0=wvc, scalar=NEG, in1=wvc,
                                op0=mybir.AluOpType.mult, op1=mybir.AluOpType.max,
                            )
                            nc.scalar.activation(
                                wvc, wvc, mybir.ActivationFunctionType.Exp)
                            # split the w*h expand-multiply between engines:
                            # gh pairs [0:GS) multiply on DVE at 1x (f32 wv
                            # broadcast); [GS:GH) broadcast-Copy on the
                            # Scalar engine into rhs, then a pure-bf16
                            # packed DVE in-place multiply at 2x_1p.
                            # Balances DVE vs Scalar so neither serializes.
                            GS = 4
                            rhs = pe.tile([128, CC * EW], BF, tag="rhs", bufs=3)
                            rhsv = rhs[:, : nk * EW].rearrange(
                                "p (c x) -> p c x", x=EW)
                            hgv = hg[:, t0 * HROW:(t0 + nk) * HROW].rearrange(
                                "p (c g f) -> p c g f", g=28, f=F)
                            wvb = wv[:, t0 * GH:(t0 + nk) * GH]\
                                .rearrange("p (c g) -> p c g", g=GH)
                            wexpv = rhsv[:, :, GS * F:G * H * F].rearrange(
                                "p c (g f) -> p c g f", f=F)
                            nc.scalar.activation(
                                wexpv,
                                wvb[:, :, GS:GH].to_broadcast([128, nk, GH - GS, F]),
                                mybir.ActivationFunctionType.Copy)
                            nc.vector.tensor_tensor(
                                out=rhsv[:, :, 0:GS * F]
                                .rearrange("p c (g f) -> p c g f", f=F),
                                in0=hgv[:, :, 0:GS, :],
                                in1=wvb[:, :, 0:GS]
                                .to_broadcast([128, nk, GS, F]),
                                op=mybir.AluOpType.mult,
                            )
                            nc.vector.tensor_tensor(
                                out=wexpv,
                                in0=hgv[:, :, GS:GH, :],
                                in1=wexpv,
                                op=mybir.AluOpType.mult,
                            )
                            nc.vector.tensor_copy(
                                rhsv[:, :, G * H * F:EW], wvb)
                            for ck in range(nk):
                                t = t0 + ck
                                first = (t == 0)
                                last = (t == KMT - 1)
                                nc.tensor.matmul(
                                    ew_ps[:, 0:512],
                                    lhsT=ind_w[:, t * 128:(t + 1) * 128],
                                    rhs=rhs[:, ck * EW: ck * EW + 512],
                                    start=first, stop=last, skip_group_check=True,
                                )
                                nc.tensor.matmul(
                                    ew_ps[:, 512:EW],
                                    lhsT=ind_w[:, t * 128:(t + 1) * 128],
                                    rhs=rhs[:, ck * EW + 512:(ck + 1) * EW],
                                    start=first, stop=last, skip_group_check=True,
                                )

                        # late prep+fire for the rotating window-3 buffer
                        # (WAR on hg0 pins desc-gen after w0's readers; the
                        # trig1b dep keeps the SWDGE ring FIFO in emission
                        # order — a w3 frame before w2c2 would be fired by
                        # the wrong trigger)
                        if w == 0:
                            for _p in gather_window(3):
                                _add_dep(_p.ins, trig1b.ins, True,
                                         "ring FIFO: w3 preps after w2c2")
                            nc.gpsimd.trigger_dma(count=None)

                        # ---- self loop + normalize + ELU(+1) for this window
                        mt = w
                        wself = wself_all[:, mt * GH:(mt + 1) * GH]
                        pseH = ew_ps[:, 0:G * H * F].rearrange(
                            "p (g f) -> p g f", f=F)
                        pseZ = ew_ps[:, G * H * F:EW].rearrange(
                            "p (g o) -> p g o", o=1)
                        nc.vector.tensor_tensor(
                            out=pseZ, in0=pseZ,
                            in1=wself.rearrange("p (g o) -> p g o", o=1),
                            op=mybir.AluOpType.add)
                        tmp = pe.tile([128, G * H * F], F32, tag="selfh", bufs=1)
                        nc.vector.tensor_tensor(
                            out=tmp[:].rearrange("p (g f) -> p g f", f=F),
                            in0=hes_sb[:, mt * HROW: mt * HROW + G * H * F]
                            .rearrange("p (g f) -> p g f", f=F),
                            in1=wself.rearrange("p (g o) -> p g o", o=1)
                            .to_broadcast([128, GH, F]),
                            op=mybir.AluOpType.mult)
                        nc.vector.tensor_tensor(
                            out=pseH, in0=pseH,
                            in1=tmp[:].rearrange("p (g f) -> p g f", f=F),
                            op=mybir.AluOpType.add)
                        zv = pe.tile([128, GH], F32, tag="zv", bufs=1)
                        nc.vector.tensor_copy(
                            zv[:].rearrange("p (g o) -> p g o", o=1), pseZ)
                        nc.vector.reciprocal(zv[:], zv[:])
                        t1 = pe.tile([128, G * H * F], F32, tag="t1", bufs=1)
                        nc.vector.tensor_tensor(
                            out=t1[:].rearrange("p (g f) -> p g f", f=F),
                            in0=pseH,
                            in1=zv[:].rearrange("p (g o) -> p g o", o=1)
                            .to_broadcast([128, GH, F]),
                            op=mybir.AluOpType.mult)
                        nc.vector.tensor_add(t1[:], t1[:], bias_sb[:])
                        t2 = pe.tile([128, G * H * F], F32, tag="t2", bufs=1)
                        nc.vector.tensor_scalar_min(t2[:], t1[:], 0.0)
                        nc.scalar.activation(t2[:], t2[:],
                                             mybir.ActivationFunctionType.Exp)
                        elu_w = pe.tile([128, G * H * F], BF, tag="elu", bufs=2)
                        nc.vector.scalar_tensor_tensor(
                            out=elu_w[:], in0=t1[:], scalar=0.0, in1=t2[:],
                            op0=mybir.AluOpType.max, op1=mybir.AluOpType.add)
                        # pair-transpose branches into s1T during the edge phase
                        for g0 in range(0, G, 2):
                            ng = min(2, G - g0)
                            t = g0 // 2
                            p_t3 = psC.tile([128, 128], BF, tag="tp3", bufs=1)
                            nc.tensor.transpose(
                                p_t3[: ng * NHID, :],
                                elu_w[:, g0 * H * F:(g0 + ng) * H * F],
                                ident[:, :],
                            )
                            nc.vector.tensor_copy(
                                s1T[: ng * NHID,
                                    t * S + mt * 128:t * S + (mt + 1) * 128],
                                p_t3[: ng * NHID, :])

                        # ---- per-window MLP + head: fills gather-paced
                        # slack in the edge phase instead of a serial tail.
                        # Block-diagonal pair weights -> one matmul per pair;
                        # ELU(+1) batched across all 7 pairs in 3 wide DVE
                        # ops (the t=6 pad rows yield exactly the constant-1
                        # column the head bias trick needs).
                        s2w = pe.tile([128, 7 * 128], BF, tag="s2w", bufs=2)
                        pm_all = psC.tile([128, 7 * 128], F32, tag="mlp", bufs=1)
                        for t in range(7):
                            nc.tensor.matmul(
                                pm_all[:, t * 128:(t + 1) * 128],
                                lhsT=mw_sb[:, t * 128:(t + 1) * 128],
                                rhs=s1T[:, t * S + mt * 128:
                                        t * S + (mt + 1) * 128],
                                start=True, stop=True, skip_group_check=True)
                        yb = pe.tile([128, 7 * 128], F32, tag="yb", bufs=2)
                        nc.vector.tensor_tensor(
                            out=yb[:].rearrange("p (t c) -> p t c", c=128),
                            in0=pm_all[:].rearrange("p (t c) -> p t c", c=128),
                            in1=mbp2_sb[:].rearrange("p (t o) -> p t o", o=1)
                            .to_broadcast([128, 7, 128]),
                            op=mybir.AluOpType.add)
                        ym = pe.tile([128, 7 * 128], F32, tag="ym", bufs=2)
                        nc.vector.tensor_scalar_min(ym[:], yb[:], 0.0)
                        nc.scalar.activation(ym[:], ym[:],
                                             mybir.ActivationFunctionType.Exp)
                        nc.vector.scalar_tensor_tensor(
                            out=s2w[:], in0=yb[:], scalar=0.0, in1=ym[:],
                            op0=mybir.AluOpType.max, op1=mybir.AluOpType.add)
                        p_fw = psC.tile([C, 128], F32, tag="fin", bufs=1)
                        for t in range(7):
                            nc.tensor.matmul(
                                p_fw[:], lhsT=outw_sb[:, t * C:(t + 1) * C],
                                rhs=s2w[:, t * 128:(t + 1) * 128],
                                start=(t == 0), stop=(t == 6))
                        nc.vector.tensor_copy(lg[:, mt * 128:(mt + 1) * 128],
                                              p_fw[:])

                # ================= log_softmax =================
                with tc.tile_pool(name="pf", bufs=1) as pf, \
                     tc.tile_pool(name="psD", bufs=1, space="PSUM") as psD:
                    # batched log_softmax over all 4 node windows
                    lt = pf.tile([128, NW * C], F32)
                    for mt in range(NW):
                        p_l = psD.tile([128, C], F32, tag="lsm", bufs=2)
                        nc.tensor.transpose(p_l[:], lg[:, mt * 128:(mt + 1) * 128],
                                            identf[:C, :C])
                        nc.vector.tensor_copy(lt[:, mt * C:(mt + 1) * C], p_l[:])
                    lt3 = lt[:].rearrange("p (m c) -> p m c", c=C)
                    mx = pf.tile([128, NW], F32)
                    nc.vector.reduce_max(mx[:].rearrange("p (m o) -> p m o", o=1),
                                         lt3, axis=mybir.AxisListType.X)
                    nc.vector.tensor_tensor(
                        out=lt3, in0=lt3,
                        in1=mx[:].rearrange("p (m o) -> p m o", o=1)
                        .to_broadcast([128, NW, C]),
                        op=mybir.AluOpType.subtract)
                    ex = pf.tile([128, NW * C], F32)
                    nc.scalar.activation(ex[:], lt[:], mybir.ActivationFunctionType.Exp)
                    se = pf.tile([128, NW], F32)
                    nc.vector.reduce_sum(se[:].rearrange("p (m o) -> p m o", o=1),
                                         ex[:].rearrange("p (m c) -> p m c", c=C),
                                         axis=mybir.AxisListType.X)
                    nc.scalar.activation(se[:], se[:], mybir.ActivationFunctionType.Ln)
                    oo = pf.tile([128, NW * C], F32)
                    nc.vector.tensor_tensor(
                        out=oo[:].rearrange("p (m c) -> p m c", c=C),
                        in0=lt3,
                        in1=se[:].rearrange("p (m o) -> p m o", o=1)
                        .to_broadcast([128, NW, C]),
                        op=mybir.AluOpType.subtract)
                    nc.sync.dma_start(
                        d_out[:].rearrange("(m p) c -> p m c", p=128),
                        oo[:].rearrange("p (m c) -> p m c", c=C))

    # re-executability: zero the gather-completion sem after Tile's teardown
    # drains (which, post-patch, wait for all prepared gathers to land)
    nc.clear_and_free_semaphores(hg_sems)
    _patch_prep_waits(nc)
    nc.compile()
    return nc


def _patch_prep_waits(nc):
    """Anything gating on a prepared gather (data consumers, SWDGE ring
    flow-control IncSwdgeSem, end-of-kernel drains) waits on Tile's DMASW
    lane sems, which nothing increments in the prepare_only flow — the DMA
    completion sem is the descriptor-baked per-lane hg_dma{L} instead.
    Retarget every DMASW{L} wait to hg_dma{L}, keeping the tick value
    (lane-local frame count * 16). Lane assignment is round-robin over the
    8 DMASW procs in scheduled order; assert it matches emission order
    (prep j -> lane j%8) via each prep's baked sem."""
    import re
    preps = [ins for ins in nc.inst_map.values()
             if type(ins).__name__ == "InstDMAGatherAnt"
             and getattr(ins, "gen_mode", 0) == 1]
    assert preps, "no prepared gathers found"
    lane_sems = {}
    for j, p in enumerate(preps):
        upd = p.sync_info.on_update[0]
        assert upd.ant_name == f"hg_dma{j % 8}", (j, upd.ant_name)
        lane_sems[j % 8] = (upd.id, upd.ant_name)
    # lane round-robin follows SCHEDULED order; the baked sems follow
    # EMISSION order — verify they coincide by walking the pool stream
    sched = []
    for b in nc.main_func.blocks:
        for ins in b.instructions:
            if (type(ins).__name__ == "InstDMAGatherAnt"
                    and getattr(ins, "gen_mode", 0) == 1):
                sched.append(ins.name)
    assert sched == [p.name for p in preps], (
        f"prep scheduled order != emission order:\n{sched}\n"
        f"{[p.name for p in preps]}")
    for ins in nc.inst_map.values():
        si = getattr(ins, "sync_info", None)
        if not si or not si.on_wait:
            continue
        for w in si.on_wait:
            m = re.match(r"DMASW(\d+)_", w.ant_name or "")
            if m and w.wait_mode == "sem-ge-imm":
                lane = int(m.group(1))
                assert lane in lane_sems, (lane, w.wait_value)
                sid, sname = lane_sems[lane]
                w.id = sid
                w.ant_name = sname


def _host_prep(inputs):
    """Shard/transpose/cast inputs; build edge structures."""
    x = np.asarray(inputs["x"], np.float32)
    edge_index = np.asarray(inputs["edge_index"]).astype(np.int64)
    U = np.asarray(inputs["U"], np.float32)
    psi = np.asarray(inputs["psi"], np.float32)
    gat_W = np.asarray(inputs["gat_W"], np.float32)
    att_src = np.asarray(inputs["att_src"], np.float32)
    att_dst = np.asarray(inputs["att_dst"], np.float32)
    gat_b = np.asarray(inputs["gat_b"], np.float32)
    mlp_W = np.asarray(inputs["mlp_W"], np.float32)
    mlp_b = np.asarray(inputs["mlp_b"], np.float32)
    out_W = np.asarray(inputs["out_W"], np.float32)
    out_b = np.asarray(inputs["out_b"], np.float32)

    src, dst = edge_index[0], edge_index[1]

    core_all = dst // S
    win_all = (dst % S) // 128
    key = core_all * NW + win_all
    order = np.argsort(key, kind="stable")
    counts = np.bincount(key, minlength=R * NW)
    maxw = counts.max()
    KMT = int((maxw + 127) // 128)
    KE = NW * KMT * 128
    TE = NW * KMT

    # shared weight packs
    wcat = np.zeros((F, G * 68), np.float32)
    for g in range(G):
        Wg = gat_W[g]                                   # [F, H*F]
        Wh = Wg.reshape(F, H, F)
        Ws = np.einsum("ihf,hf->ih", Wh, att_src[g])    # [F, H]
        Wd = np.einsum("ihf,hf->ih", Wh, att_dst[g])    # [F, H]
        wcat[:, g * 68: g * 68 + H * F] = Wg
        wcat[:, g * 68 + H * F: g * 68 + H * F + H] = Ws
        wcat[:, g * 68 + H * F + H: g * 68 + 68] = Wd
    bias = np.tile(gat_b.reshape(1, G * H * F), (128, 1)).astype(np.float32)
    # block-diagonal pair-packed MLP weights: one [128,128] matmul per pair
    mw = np.zeros((128, 7 * 128), np.float32)
    for t in range(7):
        for gg in range(min(2, G - 2 * t)):
            mw[gg * NHID:(gg + 1) * NHID,
               t * 128 + gg * NHID:t * 128 + (gg + 1) * NHID] = mlp_W[2 * t + gg]
    mbp = np.stack([mlp_b[g] - mlp_W[g].sum(0) for g in range(G)], 1)
    mbp2 = np.zeros((128, 7), np.float32)
    for g in range(G):
        mbp2[(g % 2) * NHID:(g % 2 + 1) * NHID, g // 2] = mbp[:, g]
    outw = np.zeros((7 * 128, C), np.float32)
    outw[:G * NHID, :] = out_W
    outw[G * NHID, :] = out_b - out_W.sum(0)
    outwW = _wrap128(outw)                              # [128, 7*C]

    af = np.abs(x)
    afW = _wrap128(af)                                  # [128, KT*F]

    def wrap_idx(arr):
        a = arr.reshape(-1, 16).T.astype(np.int16)
        return np.ascontiguousarray(np.tile(a, (8, 1)))

    starts = np.zeros(R * NW + 1, np.int64)
    starts[1:] = np.cumsum(counts)
    sorted_e = order

    in_maps = []
    for r in range(R):
        sl = slice(r * S, (r + 1) * S)
        # [m, kt-major] layouts: psiW[p, kt*J*S + j*S + n] = psi[j, r*S+n, kt*128+p]
        psiT = np.ascontiguousarray(
            psi[:, sl, :].transpose(2, 0, 1).reshape(N, J * S))
        psiW = _wrap128(psiT)
        uT = np.ascontiguousarray(U[sl, :].T)           # [N, S]
        uW = _wrap128(uT)

        gsrc = np.zeros(KE, np.int64)
        ldst = np.zeros(KE, np.int64)
        valid = np.zeros(KE, bool)
        for w in range(NW):
            k = r * NW + w
            es = sorted_e[starts[k]:starts[k + 1]]
            base = w * KMT * 128
            gsrc[base: base + len(es)] = src[es]
            ldst[base: base + len(es)] = dst[es] - r * S
            valid[base: base + len(es)] = True
        # window-relative dst (0..127), -1 for padding
        w_of = (np.arange(KE) // 128) // KMT
        rel = ldst - 128 * w_of
        tt = np.arange(KE) // 128
        pp = np.arange(KE) % 128
        relv = rel[valid].astype(np.int64)
        indT = np.zeros((128, TE * 128), np.float32)
        cols = tt * 128 + pp
        indT[relv, cols[valid]] = 1.0
        indF = np.zeros((128, TE * 128), np.float32)
        indF[pp[valid], tt[valid] * 128 + relv] = 1.0

        in_maps.append({
            "af": _bf(afW),
            "psiW": _bf(psiW),
            "uW": _bf(uW),
            "wcat": _bf(wcat),
            "bias": bias,
            "mw": _bf(mw),
            "mbp2": _f32(mbp2),
            "outwW": _bf(outwW),
            "gidx": wrap_idx(gsrc),
            "indF": _bf(indF),
            "indT": _bf(indT),
        })
    return in_maps, KMT, CHK


def kernel(**inputs) -> np.ndarray:
    in_maps, KMT, _ = _host_prep(inputs)
    if KMT not in _PROGRAM_CACHE:
        _PROGRAM_CACHE[KMT] = build_program(KMT)
    nc = _PROGRAM_CACHE[KMT]
    res = run_bass_kernel_spmd(nc, in_maps, list(range(R)))
    out = np.concatenate([res.results[i]["out"] for i in range(R)], axis=0)
    return out.astype(np.float32)

